# revision 32
# baseline (speedup 1.0000x reference)
"""Causal attention (B=4, N=2048, D=1024) on 8 Trainium2 NeuronCores.

Sharding: core 2b+r handles batch b, query tiles {r, r+2, ..., r+14}
(128-row tiles, parity-interleaved) — exactly balanced causal split.
K/V projections are deduplicated across the core pair: each core
projects K^T/V only for its own 8 parity-interleaved key tiles, and the
pair exchanges halves with 4 chunked AllGathers (replica groups
[[0,1],[2,3],[4,5],[6,7]]); chunk j delivers global k-tiles 4j..4j+3.
A zero-dependency dummy collective issued first absorbs the ~50us
first-collective warmup while projections run.

Everything runs in bfloat16 on the PE (fp32 PSUM accumulation), which
enables fast weight loads and halves DMA/SBUF traffic; measured
end-to-end max rel err vs the fp32 reference is ~4e-3 (gate 2e-2).
All operands (x tiles, weights, K^T, V, Q^T) are SBUF-resident.

Attention is processed in 512-column score groups that correspond 1:1
with exchange chunks: every slot's early column groups run as soon as
early chunks land (exp'd per-group with partial row sums), so the PE
always has chunk-independent work while later collectives are in
flight, and only small score tails + AV remain after the last chunk.
Per-core causal masks are passed as input data so the program is
uniform across cores (SPMD).
"""
import sys

sys.path.insert(0, "/opt/trn_rl_repo")

from contextlib import ExitStack

import numpy as np
import ml_dtypes

import concourse.bass as bass
import concourse.mybir as mybir
import concourse.tile as tile
from concourse import bacc
from concourse.bass_utils import run_bass_kernel_spmd
from concourse.masks import make_identity

B, N, D = 4, 2048, 1024
N_CORES = 8
N_SLOTS = 8          # query tiles per core
N_OWN = 8            # own key tiles per core (pair-deduplicated)
N_KT = 16            # 128-key tiles per batch
NCHUNK = 4           # collective chunks (2 own tiles -> 4 global tiles each)
SCALE = 1.0 / 32.0   # 1/sqrt(D)
NEG = -1.0e9

F32 = mybir.dt.float32
BF16 = mybir.dt.bfloat16
GROUPS = [[0, 1], [2, 3], [4, 5], [6, 7]]

_NC_CACHE = {}
TRACE = False
LAST_EXEC_NS = None


def _build_nc():
    nc = bacc.Bacc(None, target_bir_lowering=False, debug=False)

    # x own tiles, d-major: [own_tile, partition(d%128), dchunk, token]
    x_t = nc.declare_dram_parameter("x_t", [N_OWN, 128, 8, 128], BF16, isOutput=False)
    # weights: wq/wk [echunk, p(d%128), dchunk, ecol]; wv [eh, p, dchunk, ecol]
    wq = nc.declare_dram_parameter("wq", [8, 128, 8, 128], BF16, isOutput=False)
    wk = nc.declare_dram_parameter("wk", [8, 128, 8, 128], BF16, isOutput=False)
    wv = nc.declare_dram_parameter("wv", [2, 128, 8, 512], BF16, isOutput=False)
    mask_in = nc.declare_dram_parameter("mask", [128, 256], F32, isOutput=False)
    out_q = nc.declare_dram_parameter("out_q", [N_SLOTS, 128, D], F32, isOutput=True)

    with tile.TileContext(nc) as tc, ExitStack() as top:
        consts = top.enter_context(tc.tile_pool(name="consts", bufs=1))
        kt_pool = top.enter_context(tc.tile_pool(name="ktp", bufs=1))
        v_pool = top.enter_context(tc.tile_pool(name="vp", bufs=1))
        qt_pool = top.enter_context(tc.tile_pool(name="qtp", bufs=1))
        dram = top.enter_context(tc.tile_pool(name="dram", bufs=1, space="DRAM"))

        # tiny dummy collective, first thing in the program, absorbing the
        # first-collective warmup while projections run
        dummy_in = dram.tile([128, 8], F32, name="cc_dummy_in")
        dummy_out = dram.tile([2, 128, 8], F32, name="cc_dummy_out")
        nc.gpsimd.dma_start(out=dummy_in[:], in_=mask_in[:, 0:8])
        nc.gpsimd.collective_compute(
            "AllGather", mybir.AluOpType.bypass, replica_groups=GROUPS,
            ins=[dummy_in.opt()], outs=[dummy_out.opt()],
        )

        ident_f = consts.tile([128, 128], F32)
        make_identity(nc, ident_f)
        ident = consts.tile([128, 128], BF16)
        nc.vector.tensor_copy(ident, ident_f)

        KT = kt_pool.tile([128, 8, N], BF16)             # [p(e%128), echunk, global key]
        V = v_pool.tile([128, N_KT, D], BF16)            # [p(k%128), global tile, e]
        QT = qt_pool.tile([128, 8, N_SLOTS, 128], BF16)  # [p(e%128), echunk, slot, q]

        bin_t = [dram.tile([128, 4096], BF16, name=f"ccin{j}") for j in range(NCHUNK)]
        bout_t = [dram.tile([2, 128, 4096], BF16, name=f"ccout{j}") for j in range(NCHUNK)]

        mask_sb = consts.tile([128, 256], F32)

        with ExitStack() as ph1:
            xp = ph1.enter_context(tc.tile_pool(name="xp", bufs=1))
            wp = ph1.enter_context(tc.tile_pool(name="wp", bufs=1))
            stg = ph1.enter_context(tc.tile_pool(name="stg", bufs=1))
            ps_mm = ph1.enter_context(tc.tile_pool(name="ps_mm", bufs=8, space="PSUM"))

            X = xp.tile([128, N_OWN, 8, 128], BF16)      # [p, own_tile, c, q]
            nc.sync.dma_start(out=X[:, 0, 0:4, :], in_=x_t[0][:, 0:4, :])
            nc.sync.dma_start(out=X[:, 0, 4:8, :], in_=x_t[0][:, 4:8, :])
            nc.gpsimd.dma_start(out=X[:, 1, :, :], in_=x_t[1][:, :, :])
            for i in range(4, N_OWN):
                nc.sync.dma_start(out=X[:, i, :, :], in_=x_t[i][:, :, :])
            nc.sync.dma_start(out=mask_sb, in_=mask_in[:, :])

            wv_sb = wp.tile([128, 2, 8, 512], BF16)
            for eh in range(2):
                for h2 in range(2):
                    nc.scalar.dma_start(
                        out=wv_sb[:, eh, h2 * 4:(h2 + 1) * 4, :],
                        in_=wv[eh][:, h2 * 4:(h2 + 1) * 4, :],
                    )
            nc.scalar.dma_start(out=X[:, 2, :, :], in_=x_t[2][:, :, :])
            nc.scalar.dma_start(out=X[:, 3, :, :], in_=x_t[3][:, :, :])
            wk_sb = wp.tile([128, 8, 8, 128], BF16)
            wq_sb = wp.tile([128, 8, 8, 128], BF16)
            for e in range(8):
                nc.scalar.dma_start(out=wk_sb[:, e, :, :], in_=wk[e][:, :, :])
            for e in range(8):
                nc.scalar.dma_start(out=wq_sb[:, e, :, :], in_=wq[e][:, :, :])

            # staging for own-half projections (own-local layout, rank-uniform)
            KTstg = stg.tile([128, 8, N_OWN, 128], BF16)  # [p(e%128), echunk, i, key]
            Vstg = stg.tile([128, N_OWN, D], BF16)        # [p(k%128), i, e]

            def v_tile(i):
                for eh in range(2):
                    vps = ps_mm.tile([128, 512], F32, tag="mm", name=f"v{i}_{eh}")
                    for c in range(8):
                        nc.tensor.matmul(
                            vps, X[:, i, c, :], wv_sb[:, eh, c, :],
                            start=(c == 0), stop=(c == 7),
                        )
                    nc.vector.tensor_copy(Vstg[:, i, eh * 512:(eh + 1) * 512], vps)

            def k_group(kg):
                for e in range(8):
                    kps = ps_mm.tile([128, 512], F32, tag="mm", name=f"k{kg}_{e}")
                    for c in range(8):
                        nc.tensor.matmul(
                            kps, wk_sb[:, e, c, :], X[:, kg * 4:(kg + 1) * 4, c, :],
                            start=(c == 0), stop=(c == 7),
                        )
                    nc.vector.tensor_copy(
                        KTstg[:, e, kg * 4:(kg + 1) * 4, :],
                        kps.rearrange("p (i q) -> p i q", i=4),
                    )

            def send_chunk(j):
                nc.gpsimd.dma_start(
                    out=bin_t[j][:, 0:2048],
                    in_=KTstg[:, :, 2 * j:2 * j + 2, :],
                )
                nc.gpsimd.dma_start(
                    out=bin_t[j][:, 2048:4096],
                    in_=Vstg[:, 2 * j:2 * j + 2, :].rearrange("p i e -> p (i e)"),
                )
                nc.gpsimd.collective_compute(
                    "AllGather", mybir.AluOpType.bypass, replica_groups=GROUPS,
                    ins=[bin_t[j].opt()], outs=[bout_t[j].opt()],
                )

            # bin K part is (e, i2, q); global tile g = 4j + 2*i2 + rk
            V_i = V.rearrange("p (t2 r) v -> p t2 r v", r=2)

            def recv_chunk(j):
                for i2 in range(2):  # KT first (S needs it), low tiles first
                    for rk in range(2):
                        g = 4 * j + 2 * i2 + rk
                        nc.sync.dma_start(
                            out=KT[:, :, g * 128:(g + 1) * 128],
                            in_=bout_t[j][rk, :, 0:2048]
                            .rearrange("p (e i q) -> p e i q", e=8, i=2)[:, :, i2, :],
                        )
                for rk in range(2):
                    nc.sync.dma_start(
                        out=V_i[:, 2 * j:2 * j + 2, rk, :],
                        in_=bout_t[j][rk, :, 2048:4096]
                        .rearrange("p (i v) -> p i v", i=2),
                    )

            def q_group(qg):
                for e in range(8):
                    qps = ps_mm.tile([128, 512], F32, tag="mm", name=f"q{qg}_{e}")
                    for c in range(8):
                        nc.tensor.matmul(
                            qps, wq_sb[:, e, c, :], X[:, qg * 4:(qg + 1) * 4, c, :],
                            start=(c == 0), stop=(c == 7),
                        )
                    nc.vector.tensor_copy(
                        QT[:, e, qg * 4:(qg + 1) * 4, :],
                        qps.rearrange("p (s q) -> p s q", s=4),
                    )

            # KV chunks 0/1 first (they start the serial collective chain),
            # then Q so attention can start as soon as chunk 0 lands; KV
            # chunks 2/3 last (the collective engine is busy until then anyway)
            v_tile(0); v_tile(1)
            k_group(0)
            send_chunk(0)
            k_group(1)
            v_tile(2); v_tile(3)
            send_chunk(1)
            v_tile(4); v_tile(5)
            send_chunk(2)
            q_group(0)
            v_tile(6); v_tile(7)
            send_chunk(3)
            q_group(1)
            for j in range(NCHUNK):
                recv_chunk(j)

        # ---- attention: 512-col score groups paced by chunk arrival ----
        with ExitStack() as ph3:
            ps_s = ph3.enter_context(tc.tile_pool(name="ps_s", bufs=3, space="PSUM"))
            ps_tr = ph3.enter_context(tc.tile_pool(name="ps_tr", bufs=3, space="PSUM"))
            ps_o = ph3.enter_context(tc.tile_pool(name="ps_o", bufs=1, space="PSUM"))
            p_pool = ph3.enter_context(tc.tile_pool(name="pp", bufs=8))
            pt_pool = ph3.enter_context(tc.tile_pool(name="ptp", bufs=3))
            sc_pool = ph3.enter_context(tc.tile_pool(name="scp", bufs=8))
            outp = ph3.enter_context(tc.tile_pool(name="outp", bufs=2))
            accp = ph3.enter_context(tc.tile_pool(name="accp", bufs=2))

            P = {}
            ST = {}
            for i in range(N_SLOTS):
                L = 2 * (i + 1)
                P[i] = p_pool.tile([128, L * 128], BF16, tag="P", name=f"P{i}")
                ST[i] = sc_pool.tile([128, 8], F32, tag="st", name=f"st{i}")

            def ngroups(i):
                return (2 * (i + 1) * 128 + 511) // 512

            def s_part(i, kg):
                """Scores for slot i, columns [kg*512, ...): needs KT chunk kg."""
                L = 2 * (i + 1)
                lo = kg * 512
                w = min(512, L * 128 - lo)
                sp = ps_s.tile([128, w], F32, tag="S", name=f"S{i}_{kg}")
                for e in range(8):
                    nc.tensor.matmul(
                        sp, QT[:, e, i, :], KT[:, e, lo:lo + w],
                        start=(e == 0), stop=(e == 7),
                    )
                if lo + w == L * 128:
                    nc.vector.tensor_add(sp[:, w - 256:w], sp[:, w - 256:w], mask_sb)
                # scores/32 are bounded (|s|/32 <~ 11) -> exp w/o max-subtraction
                nc.scalar.activation(
                    P[i][:, lo:lo + w], sp,
                    mybir.ActivationFunctionType.Exp,
                    bias=0.0, scale=SCALE, accum_out=ST[i][:, kg:kg + 1],
                )

            def av_tiles(i, O_ps, k0, k1, L):
                for kt in range(k0, k1):
                    ptps = ps_tr.tile([128, 128], BF16, tag="tr", name=f"tp{i}_{kt}")
                    nc.tensor.transpose(ptps, P[i][:, kt * 128:(kt + 1) * 128], ident)
                    pt_sb = pt_pool.tile([128, 128], BF16, tag="pts", name=f"pt{i}_{kt}")
                    nc.vector.tensor_copy(pt_sb, ptps)
                    for h in range(2):
                        nc.tensor.matmul(
                            O_ps[:, h * 512:(h + 1) * 512], pt_sb,
                            V[:, kt, h * 512:(h + 1) * 512],
                            start=(kt == k0), stop=(kt == k1 - 1),
                        )

            def rowsum_recip(i):
                ng = ngroups(i)
                stats = ST[i]
                acc = stats[:, 0:1]
                col = 4
                for kg in range(1, ng):
                    nxt = stats[:, col:col + 1]
                    nc.vector.tensor_add(nxt, acc, stats[:, kg:kg + 1])
                    acc = nxt
                    col += 1
                recip = stats[:, 7:8]
                nc.vector.reciprocal(recip, acc)
                return recip

            def emit_av(i):
                L = 2 * (i + 1)
                O_ps = ps_o.tile([128, D], F32, tag="O", name=f"O{i}")
                av_tiles(i, O_ps, 0, L, L)
                recip = rowsum_recip(i)
                out_sb = outp.tile([128, D], F32, tag="osb", name=f"ou{i}")
                nc.vector.tensor_scalar_mul(out_sb, O_ps, recip)
                nc.scalar.dma_start(out=out_q[i][:, :], in_=out_sb)

            O_acc = {}

            def emit_av_acc(i, k0, k1):
                """AV over k-tiles [k0, k1) accumulated in an SBUF f32 tile."""
                O_ps = ps_o.tile([128, D], F32, tag="O", name=f"Oh{i}_{k0}")
                av_tiles(i, O_ps, k0, k1, k1)
                if k0 == 0:
                    O_acc[i] = accp.tile([128, D], F32, tag="acc", name=f"acc{i}")
                    nc.vector.tensor_copy(O_acc[i], O_ps)
                else:
                    nc.vector.tensor_add(O_acc[i], O_acc[i], O_ps)

            def emit_av_tail(i, khead):
                L = 2 * (i + 1)
                O_ps = ps_o.tile([128, D], F32, tag="O", name=f"Ot{i}")
                av_tiles(i, O_ps, khead, L, L)
                recip = rowsum_recip(i)
                out_sb = outp.tile([128, D], F32, tag="osb", name=f"ou{i}")
                nc.vector.tensor_add(out_sb, O_acc[i], O_ps)
                nc.vector.tensor_scalar_mul(out_sb, out_sb, recip)
                nc.scalar.dma_start(out=out_q[i][:, :], in_=out_sb)

            # chunk 0/1 work (those chunks land early); later groups deferred
            s_part(0, 0)
            s_part(1, 0)
            s_part(2, 0); s_part(2, 1)
            emit_av(0)
            s_part(3, 0); s_part(3, 1)
            emit_av(1)
            s_part(4, 0); s_part(4, 1)
            emit_av(2)
            s_part(5, 0); s_part(5, 1)
            emit_av(3)
            s_part(6, 0); s_part(6, 1)
            s_part(7, 0); s_part(7, 1)
            emit_av_acc(6, 0, 8)   # V chunks 0/1: fill the chunk-2 wait
            emit_av_acc(7, 0, 8)
            # chunk 2 work
            s_part(4, 2)
            emit_av(4)
            s_part(5, 2)
            emit_av(5)
            s_part(6, 2)
            s_part(7, 2)
            emit_av_acc(6, 8, 12)
            emit_av_acc(7, 8, 12)
            # chunk 3 work: only 512-col score tails + short AV tails remain
            s_part(6, 3)
            s_part(7, 3)
            emit_av_tail(6, 12)
            emit_av_tail(7, 12)

    nc.compile()
    return nc


def _masks():
    q = np.arange(128)[:, None]
    k = np.arange(128)[None, :]
    tril_add = np.where(k <= q, 0.0, NEG).astype(np.float32)
    m0 = np.concatenate([tril_add, np.full((128, 128), NEG, np.float32)], axis=1)
    m1 = np.concatenate([np.zeros((128, 128), np.float32), tril_add], axis=1)
    return m0, m1


def kernel(x, Wq, Wk, Wv):
    global LAST_EXEC_NS
    x = np.ascontiguousarray(np.asarray(x, dtype=np.float32))
    Wq = np.ascontiguousarray(np.asarray(Wq, dtype=np.float32))
    Wk = np.ascontiguousarray(np.asarray(Wk, dtype=np.float32))
    Wv = np.ascontiguousarray(np.asarray(Wv, dtype=np.float32))

    if "nc" not in _NC_CACHE:
        _NC_CACHE["nc"] = _build_nc()
    nc = _NC_CACHE["nc"]

    bf = ml_dtypes.bfloat16
    # host pre-transpose: x[b] (N, D) -> (tile, p=d%128, dchunk, token)
    xt_all = np.ascontiguousarray(
        x.reshape(B, N_KT, 128, 8, 128).transpose(0, 1, 4, 3, 2).astype(bf)
    )  # [B, tile, p, c, q]

    wq_r = np.ascontiguousarray(Wq.reshape(8, 128, 8, 128).transpose(2, 1, 0, 3).astype(bf))
    wk_r = np.ascontiguousarray(Wk.reshape(8, 128, 8, 128).transpose(2, 1, 0, 3).astype(bf))
    wv_r = np.ascontiguousarray(Wv.reshape(8, 128, 2, 512).transpose(2, 1, 0, 3).astype(bf))

    m0, m1 = _masks()
    in_maps = []
    for c in range(N_CORES):
        b, par = divmod(c, 2)
        in_maps.append({
            "x_t": np.ascontiguousarray(xt_all[b, par::2]),
            "wq": wq_r, "wk": wk_r, "wv": wv_r,
            "mask": m1 if par else m0,
        })

    res = run_bass_kernel_spmd(nc, in_maps, list(range(N_CORES)), trace=TRACE)
    LAST_EXEC_NS = res.exec_time_ns

    out = np.empty((B, N, D), dtype=np.float32)
    for c in range(N_CORES):
        b, par = divmod(c, 2)
        oq = res.results[c]["out_q"]
        for i in range(N_SLOTS):
            g = 2 * i + par
            out[b, g * 128:(g + 1) * 128, :] = oq[i]
    return out


# revision 33
# speedup vs baseline: 1.0674x; 1.0674x over previous
"""Causal attention (B=4, N=2048, D=1024) on 8 Trainium2 NeuronCores.

Sharding: core 2b+r handles batch b, query tiles {r, r+2, ..., r+14}
(128-row tiles, parity-interleaved) — exactly balanced causal split.
K/V projections are deduplicated across the core pair: each core
projects K^T/V only for its own 8 parity-interleaved key tiles, and the
pair exchanges halves with 4 chunked AllGathers (replica groups
[[0,1],[2,3],[4,5],[6,7]]); chunk j delivers global k-tiles 4j..4j+3.
A zero-dependency dummy collective issued first absorbs the ~50us
first-collective warmup while projections run.

Everything runs in bfloat16 on the PE (fp32 PSUM accumulation), which
enables fast weight loads and halves DMA/SBUF traffic; measured
end-to-end max rel err vs the fp32 reference is ~4e-3 (gate 2e-2).
All operands (x tiles, weights, K^T, V, Q^T) are SBUF-resident.

Attention is processed in 512-column score groups that correspond 1:1
with exchange chunks: every slot's early column groups run as soon as
early chunks land (exp'd per-group with partial row sums), so the PE
always has chunk-independent work while later collectives are in
flight, and only small score tails + AV remain after the last chunk.
Per-core causal masks are passed as input data so the program is
uniform across cores (SPMD).
"""
import sys

sys.path.insert(0, "/opt/trn_rl_repo")

from contextlib import ExitStack

import numpy as np
import ml_dtypes

import concourse.bass as bass
import concourse.mybir as mybir
import concourse.tile as tile
from concourse import bacc
from concourse.bass_utils import run_bass_kernel_spmd
from concourse.masks import make_identity

B, N, D = 4, 2048, 1024
N_CORES = 8
N_SLOTS = 8          # query tiles per core
N_OWN = 8            # own key tiles per core (pair-deduplicated)
N_KT = 16            # 128-key tiles per batch
NCHUNK = 4           # collective chunks (2 own tiles -> 4 global tiles each)
SCALE = 1.0 / 32.0   # 1/sqrt(D)
NEG = -1.0e9

F32 = mybir.dt.float32
BF16 = mybir.dt.bfloat16
GROUPS = [[0, 1], [2, 3], [4, 5], [6, 7]]

_NC_CACHE = {}
TRACE = False
LAST_EXEC_NS = None


def _build_nc():
    nc = bacc.Bacc(None, target_bir_lowering=False, debug=False)

    # x own tiles, d-major: [own_tile, partition(d%128), dchunk, token]
    x_t = nc.declare_dram_parameter("x_t", [N_OWN, 128, 8, 128], BF16, isOutput=False)
    # weights: wq/wk [echunk, p(d%128), dchunk, ecol]; wv [eh, p, dchunk, ecol]
    wq = nc.declare_dram_parameter("wq", [8, 128, 8, 128], BF16, isOutput=False)
    wk = nc.declare_dram_parameter("wk", [8, 128, 8, 128], BF16, isOutput=False)
    wv = nc.declare_dram_parameter("wv", [2, 128, 8, 512], BF16, isOutput=False)
    mask_in = nc.declare_dram_parameter("mask", [128, 256], F32, isOutput=False)
    out_q = nc.declare_dram_parameter("out_q", [N_SLOTS, 128, D], F32, isOutput=True)

    with tile.TileContext(nc) as tc, ExitStack() as top:
        consts = top.enter_context(tc.tile_pool(name="consts", bufs=1))
        kt_pool = top.enter_context(tc.tile_pool(name="ktp", bufs=1))
        v_pool = top.enter_context(tc.tile_pool(name="vp", bufs=1))
        qt_pool = top.enter_context(tc.tile_pool(name="qtp", bufs=1))
        dram = top.enter_context(tc.tile_pool(name="dram", bufs=1, space="DRAM"))

        # tiny dummy collective, first thing in the program, absorbing the
        # first-collective warmup while projections run
        dummy_in = dram.tile([128, 8], F32, name="cc_dummy_in")
        dummy_out = dram.tile([2, 128, 8], F32, name="cc_dummy_out")
        nc.gpsimd.dma_start(out=dummy_in[:], in_=mask_in[:, 0:8])
        nc.gpsimd.collective_compute(
            "AllGather", mybir.AluOpType.bypass, replica_groups=GROUPS,
            ins=[dummy_in.opt()], outs=[dummy_out.opt()],
        )

        ident_f = consts.tile([128, 128], F32)
        make_identity(nc, ident_f)
        ident = consts.tile([128, 128], BF16)
        nc.vector.tensor_copy(ident, ident_f)

        KT = kt_pool.tile([128, 8, N], BF16)             # [p(e%128), echunk, global key]
        V = v_pool.tile([128, N_KT, D], BF16)            # [p(k%128), global tile, e]
        QT = qt_pool.tile([128, 8, N_SLOTS, 128], BF16)  # [p(e%128), echunk, slot, q]

        bin_t = [dram.tile([128, 4096], BF16, name=f"ccin{j}") for j in range(NCHUNK)]
        bout_t = [dram.tile([2, 128, 4096], BF16, name=f"ccout{j}") for j in range(NCHUNK)]

        mask_sb = consts.tile([128, 256], F32)

        with ExitStack() as ph1:
            xp = ph1.enter_context(tc.tile_pool(name="xp", bufs=1))
            wp = ph1.enter_context(tc.tile_pool(name="wp", bufs=1))
            stg = ph1.enter_context(tc.tile_pool(name="stg", bufs=1))
            ps_mm = ph1.enter_context(tc.tile_pool(name="ps_mm", bufs=8, space="PSUM"))

            X = xp.tile([128, N_OWN, 8, 128], BF16)      # [p, own_tile, c, q]
            nc.sync.dma_start(out=X[:, 0, 0:4, :], in_=x_t[0][:, 0:4, :])
            nc.sync.dma_start(out=X[:, 0, 4:8, :], in_=x_t[0][:, 4:8, :])
            nc.gpsimd.dma_start(out=X[:, 1, :, :], in_=x_t[1][:, :, :])
            for i in range(4, N_OWN):
                nc.sync.dma_start(out=X[:, i, :, :], in_=x_t[i][:, :, :])
            nc.sync.dma_start(out=mask_sb, in_=mask_in[:, :])

            wv_sb = wp.tile([128, 2, 8, 512], BF16)
            for eh in range(2):
                for h2 in range(2):
                    nc.scalar.dma_start(
                        out=wv_sb[:, eh, h2 * 4:(h2 + 1) * 4, :],
                        in_=wv[eh][:, h2 * 4:(h2 + 1) * 4, :],
                    )
            nc.scalar.dma_start(out=X[:, 2, :, :], in_=x_t[2][:, :, :])
            nc.scalar.dma_start(out=X[:, 3, :, :], in_=x_t[3][:, :, :])
            wk_sb = wp.tile([128, 8, 8, 128], BF16)
            wq_sb = wp.tile([128, 8, 8, 128], BF16)
            for e in range(8):
                nc.scalar.dma_start(out=wk_sb[:, e, :, :], in_=wk[e][:, :, :])
            for e in range(8):
                nc.scalar.dma_start(out=wq_sb[:, e, :, :], in_=wq[e][:, :, :])

            # staging for own-half projections (own-local layout, rank-uniform)
            KTstg = stg.tile([128, 8, N_OWN, 128], BF16)  # [p(e%128), echunk, i, key]
            Vstg = stg.tile([128, N_OWN, D], BF16)        # [p(k%128), i, e]

            def v_tile(i):
                for eh in range(2):
                    vps = ps_mm.tile([128, 512], F32, tag="mm", name=f"v{i}_{eh}")
                    for c in range(8):
                        nc.tensor.matmul(
                            vps, X[:, i, c, :], wv_sb[:, eh, c, :],
                            start=(c == 0), stop=(c == 7),
                        )
                    nc.vector.tensor_copy(Vstg[:, i, eh * 512:(eh + 1) * 512], vps)

            def k_group(kg):
                for e in range(8):
                    kps = ps_mm.tile([128, 512], F32, tag="mm", name=f"k{kg}_{e}")
                    for c in range(8):
                        nc.tensor.matmul(
                            kps, wk_sb[:, e, c, :], X[:, kg * 4:(kg + 1) * 4, c, :],
                            start=(c == 0), stop=(c == 7),
                        )
                    nc.vector.tensor_copy(
                        KTstg[:, e, kg * 4:(kg + 1) * 4, :],
                        kps.rearrange("p (i q) -> p i q", i=4),
                    )

            def send_chunk(j):
                nc.gpsimd.dma_start(
                    out=bin_t[j][:, 0:2048],
                    in_=KTstg[:, :, 2 * j:2 * j + 2, :],
                )
                nc.gpsimd.dma_start(
                    out=bin_t[j][:, 2048:4096],
                    in_=Vstg[:, 2 * j:2 * j + 2, :].rearrange("p i e -> p (i e)"),
                )
                nc.gpsimd.collective_compute(
                    "AllGather", mybir.AluOpType.bypass, replica_groups=GROUPS,
                    ins=[bin_t[j].opt()], outs=[bout_t[j].opt()],
                )

            # bin K part is (e, i2, q); global tile g = 4j + 2*i2 + rk
            V_i = V.rearrange("p (t2 r) v -> p t2 r v", r=2)

            def recv_chunk(j):
                for i2 in range(2):  # KT first (S needs it), low tiles first
                    for rk in range(2):
                        g = 4 * j + 2 * i2 + rk
                        nc.sync.dma_start(
                            out=KT[:, :, g * 128:(g + 1) * 128],
                            in_=bout_t[j][rk, :, 0:2048]
                            .rearrange("p (e i q) -> p e i q", e=8, i=2)[:, :, i2, :],
                        )
                for rk in range(2):
                    nc.sync.dma_start(
                        out=V_i[:, 2 * j:2 * j + 2, rk, :],
                        in_=bout_t[j][rk, :, 2048:4096]
                        .rearrange("p (i v) -> p i v", i=2),
                    )

            def q_group(qg):
                for e in range(8):
                    qps = ps_mm.tile([128, 512], F32, tag="mm", name=f"q{qg}_{e}")
                    for c in range(8):
                        nc.tensor.matmul(
                            qps, wq_sb[:, e, c, :], X[:, qg * 4:(qg + 1) * 4, c, :],
                            start=(c == 0), stop=(c == 7),
                        )
                    nc.vector.tensor_copy(
                        QT[:, e, qg * 4:(qg + 1) * 4, :],
                        qps.rearrange("p (s q) -> p s q", s=4),
                    )

            # KV chunks 0/1 first (they start the serial collective chain),
            # then Q so attention can start as soon as chunk 0 lands; KV
            # chunks 2/3 last (the collective engine is busy until then anyway)
            v_tile(0); v_tile(1)
            k_group(0)
            send_chunk(0)
            k_group(1)
            v_tile(2); v_tile(3)
            send_chunk(1)
            v_tile(4); v_tile(5)
            send_chunk(2)
            v_tile(6); v_tile(7)
            send_chunk(3)
            q_group(0)
            q_group(1)
            for j in range(NCHUNK):
                recv_chunk(j)

        # ---- attention: 512-col score groups paced by chunk arrival ----
        with ExitStack() as ph3:
            ps_s = ph3.enter_context(tc.tile_pool(name="ps_s", bufs=3, space="PSUM"))
            ps_tr = ph3.enter_context(tc.tile_pool(name="ps_tr", bufs=3, space="PSUM"))
            ps_o = ph3.enter_context(tc.tile_pool(name="ps_o", bufs=1, space="PSUM"))
            p_pool = ph3.enter_context(tc.tile_pool(name="pp", bufs=8))
            pt_pool = ph3.enter_context(tc.tile_pool(name="ptp", bufs=3))
            sc_pool = ph3.enter_context(tc.tile_pool(name="scp", bufs=8))
            outp = ph3.enter_context(tc.tile_pool(name="outp", bufs=2))
            accp = ph3.enter_context(tc.tile_pool(name="accp", bufs=2))

            P = {}
            ST = {}
            for i in range(N_SLOTS):
                L = 2 * (i + 1)
                P[i] = p_pool.tile([128, L * 128], BF16, tag="P", name=f"P{i}")
                ST[i] = sc_pool.tile([128, 8], F32, tag="st", name=f"st{i}")

            def ngroups(i):
                return (2 * (i + 1) * 128 + 511) // 512

            def s_part(i, kg):
                """Scores for slot i, columns [kg*512, ...): needs KT chunk kg."""
                L = 2 * (i + 1)
                lo = kg * 512
                w = min(512, L * 128 - lo)
                sp = ps_s.tile([128, w], F32, tag="S", name=f"S{i}_{kg}")
                for e in range(8):
                    nc.tensor.matmul(
                        sp, QT[:, e, i, :], KT[:, e, lo:lo + w],
                        start=(e == 0), stop=(e == 7),
                    )
                if lo + w == L * 128:
                    nc.vector.tensor_add(sp[:, w - 256:w], sp[:, w - 256:w], mask_sb)
                # scores/32 are bounded (|s|/32 <~ 11) -> exp w/o max-subtraction
                nc.scalar.activation(
                    P[i][:, lo:lo + w], sp,
                    mybir.ActivationFunctionType.Exp,
                    bias=0.0, scale=SCALE, accum_out=ST[i][:, kg:kg + 1],
                )

            def av_tiles(i, O_ps, k0, k1, L):
                for kt in range(k0, k1):
                    ptps = ps_tr.tile([128, 128], BF16, tag="tr", name=f"tp{i}_{kt}")
                    nc.tensor.transpose(ptps, P[i][:, kt * 128:(kt + 1) * 128], ident)
                    pt_sb = pt_pool.tile([128, 128], BF16, tag="pts", name=f"pt{i}_{kt}")
                    nc.vector.tensor_copy(pt_sb, ptps)
                    for h in range(2):
                        nc.tensor.matmul(
                            O_ps[:, h * 512:(h + 1) * 512], pt_sb,
                            V[:, kt, h * 512:(h + 1) * 512],
                            start=(kt == k0), stop=(kt == k1 - 1),
                        )

            def rowsum_recip(i):
                ng = ngroups(i)
                stats = ST[i]
                acc = stats[:, 0:1]
                col = 4
                for kg in range(1, ng):
                    nxt = stats[:, col:col + 1]
                    nc.vector.tensor_add(nxt, acc, stats[:, kg:kg + 1])
                    acc = nxt
                    col += 1
                recip = stats[:, 7:8]
                nc.vector.reciprocal(recip, acc)
                return recip

            def emit_av(i):
                L = 2 * (i + 1)
                O_ps = ps_o.tile([128, D], F32, tag="O", name=f"O{i}")
                av_tiles(i, O_ps, 0, L, L)
                recip = rowsum_recip(i)
                out_sb = outp.tile([128, D], F32, tag="osb", name=f"ou{i}")
                nc.vector.tensor_scalar_mul(out_sb, O_ps, recip)
                nc.scalar.dma_start(out=out_q[i][:, :], in_=out_sb)

            O_acc = {}

            def emit_av_acc(i, k0, k1):
                """AV over k-tiles [k0, k1) accumulated in an SBUF f32 tile."""
                O_ps = ps_o.tile([128, D], F32, tag="O", name=f"Oh{i}_{k0}")
                av_tiles(i, O_ps, k0, k1, k1)
                if k0 == 0:
                    O_acc[i] = accp.tile([128, D], F32, tag="acc", name=f"acc{i}")
                    nc.vector.tensor_copy(O_acc[i], O_ps)
                else:
                    nc.vector.tensor_add(O_acc[i], O_acc[i], O_ps)

            def emit_av_tail(i, khead):
                L = 2 * (i + 1)
                O_ps = ps_o.tile([128, D], F32, tag="O", name=f"Ot{i}")
                av_tiles(i, O_ps, khead, L, L)
                recip = rowsum_recip(i)
                out_sb = outp.tile([128, D], F32, tag="osb", name=f"ou{i}")
                nc.vector.tensor_add(out_sb, O_acc[i], O_ps)
                nc.vector.tensor_scalar_mul(out_sb, out_sb, recip)
                nc.scalar.dma_start(out=out_q[i][:, :], in_=out_sb)

            # chunk 0/1 work (those chunks land early); later groups deferred
            s_part(0, 0)
            s_part(1, 0)
            s_part(2, 0); s_part(2, 1)
            emit_av(0)
            s_part(3, 0); s_part(3, 1)
            emit_av(1)
            s_part(4, 0); s_part(4, 1)
            emit_av(2)
            s_part(5, 0); s_part(5, 1)
            emit_av(3)
            s_part(6, 0); s_part(6, 1)
            s_part(7, 0); s_part(7, 1)
            emit_av_acc(6, 0, 8)   # V chunks 0/1: fill the chunk-2 wait
            emit_av_acc(7, 0, 8)
            # chunk 2 work
            s_part(4, 2)
            emit_av(4)
            s_part(5, 2)
            emit_av(5)
            s_part(6, 2)
            s_part(7, 2)
            emit_av_acc(6, 8, 12)
            emit_av_acc(7, 8, 12)
            # chunk 3 work: only 512-col score tails + short AV tails remain
            s_part(6, 3)
            s_part(7, 3)
            emit_av_tail(6, 12)
            emit_av_tail(7, 12)

    nc.compile()
    return nc


def _masks():
    q = np.arange(128)[:, None]
    k = np.arange(128)[None, :]
    tril_add = np.where(k <= q, 0.0, NEG).astype(np.float32)
    m0 = np.concatenate([tril_add, np.full((128, 128), NEG, np.float32)], axis=1)
    m1 = np.concatenate([np.zeros((128, 128), np.float32), tril_add], axis=1)
    return m0, m1


def kernel(x, Wq, Wk, Wv):
    global LAST_EXEC_NS
    x = np.ascontiguousarray(np.asarray(x, dtype=np.float32))
    Wq = np.ascontiguousarray(np.asarray(Wq, dtype=np.float32))
    Wk = np.ascontiguousarray(np.asarray(Wk, dtype=np.float32))
    Wv = np.ascontiguousarray(np.asarray(Wv, dtype=np.float32))

    if "nc" not in _NC_CACHE:
        _NC_CACHE["nc"] = _build_nc()
    nc = _NC_CACHE["nc"]

    bf = ml_dtypes.bfloat16
    # host pre-transpose: x[b] (N, D) -> (tile, p=d%128, dchunk, token)
    xt_all = np.ascontiguousarray(
        x.reshape(B, N_KT, 128, 8, 128).transpose(0, 1, 4, 3, 2).astype(bf)
    )  # [B, tile, p, c, q]

    wq_r = np.ascontiguousarray(Wq.reshape(8, 128, 8, 128).transpose(2, 1, 0, 3).astype(bf))
    wk_r = np.ascontiguousarray(Wk.reshape(8, 128, 8, 128).transpose(2, 1, 0, 3).astype(bf))
    wv_r = np.ascontiguousarray(Wv.reshape(8, 128, 2, 512).transpose(2, 1, 0, 3).astype(bf))

    m0, m1 = _masks()
    in_maps = []
    for c in range(N_CORES):
        b, par = divmod(c, 2)
        in_maps.append({
            "x_t": np.ascontiguousarray(xt_all[b, par::2]),
            "wq": wq_r, "wk": wk_r, "wv": wv_r,
            "mask": m1 if par else m0,
        })

    res = run_bass_kernel_spmd(nc, in_maps, list(range(N_CORES)), trace=TRACE)
    LAST_EXEC_NS = res.exec_time_ns

    out = np.empty((B, N, D), dtype=np.float32)
    for c in range(N_CORES):
        b, par = divmod(c, 2)
        oq = res.results[c]["out_q"]
        for i in range(N_SLOTS):
            g = 2 * i + par
            out[b, g * 128:(g + 1) * 128, :] = oq[i]
    return out
